# revision 3
# baseline (speedup 1.0000x reference)
"""Trainium2 Bass kernel for nn_NeighborPointsGenerator.

Data-parallel over (image, x-slab): 8 cores = 4 images x 2 x-slabs of 128 rows.
Per core:
  - disk/negative flag: per x-row point lists (host-binned, |gx-x|<=20.01),
    slot loop computing the reference's exact fp32 rounding chain:
      c2 = fl(2*gy*y + 2*fl(x*gx))      (ScalarE FMA, bit-exact vs XLA cpu)
      d2 = fl(fl(p2+g2) - c2)           (GPSIMD add + DVE subtract)
      M  = min(M, d2)                   (DVE)
    negative  <=>  M >= nextafter(nextafter(400))   (sqrt-free equivalence)
  - ring/positive flag: base indices scattered into a DRAM window
    (gpsimd local_scatter), reloaded via one overlapping 3D-AP DMA, and
    120 shifted slices accumulated on the PE into PSUM; ring = sum > 0.
  - flag = max(ring ? 1 : -1, -(M >= T2)) ; pixels_out copied from constants.
"""
import numpy as np
import ml_dtypes

import concourse.bass as bass
import concourse.bacc as bacc
import concourse.mybir as mybir
import concourse.tile as tile
from concourse.bass_utils import run_bass_kernel_spmd
from concourse._compat import get_trn_type

F32 = mybir.dt.float32
BF16 = mybir.dt.bfloat16
I16 = mybir.dt.int16

RADIUS, STRIDE, BASE_PTS = 5, 4, 8
BS, H, W = 4, 256, 256
HW = H * W
P = 128                  # partitions = x-rows per core
N_CORES = 8
R_CULL = 20.01           # |gx - x| beyond this can never give ref-d2 <= thr

_nextafter = np.nextafter
T2 = float(_nextafter(_nextafter(np.float32(400.0), np.float32(1e9)),
                      np.float32(1e9)))  # negative <=> min_d2 >= T2


def _ring_offsets():
    dxs, dys = [], []
    for i in range(RADIUS):
        r = (i + 1) * STRIDE
        n = BASE_PTS * (i + 1)
        ang = np.arange(n) / n * 2.0 * np.pi
        dxs.append(r * np.cos(ang))
        dys.append(r * np.sin(ang))
    dx = np.concatenate(dxs)
    dy = np.concatenate(dys)
    return np.round(dy * W + dx * H).astype(np.int32)  # (K,)


_prog_cache = {}


def _build_program(pmax, nidx, wfree, na, offs_ab):
    key = (pmax, nidx, wfree, na, tuple(offs_ab))
    if key in _prog_cache:
        return _prog_cache[key]

    nc = bacc.Bacc(get_trn_type() or "TRN2", target_bir_lowering=False, debug=True)

    g2s = nc.declare_dram_parameter("g2s", [P, pmax], F32, isOutput=False)
    t1s = nc.declare_dram_parameter("t1s", [P, pmax], F32, isOutput=False)
    gy2s = nc.declare_dram_parameter("gy2s", [P, pmax], F32, isOutput=False)
    p2c = nc.declare_dram_parameter("p2c", [P, 256], F32, isOutput=False)
    ybc = nc.declare_dram_parameter("ybc", [P, 256], F32, isOutput=False)
    pxc = nc.declare_dram_parameter("pxc", [P, 512], F32, isOutput=False)
    identb = nc.declare_dram_parameter("identb", [P, P], BF16, isOutput=False)
    identf = nc.declare_dram_parameter("identf", [P, P], F32, isOutput=False)
    identfn = nc.declare_dram_parameter("identfn", [P, P], F32, isOutput=False)
    lsi = nc.declare_dram_parameter("lsi", [P, nidx], I16, isOutput=False)
    lsd = nc.declare_dram_parameter("lsd", [P, nidx], BF16, isOutput=False)

    flag_out = nc.declare_dram_parameter("flag_out", [P, 256], F32, isOutput=True)
    px_out = nc.declare_dram_parameter("px_out", [P, 512], F32, isOutput=True)

    pwin = nc.dram_tensor("pwin", [P, wfree], BF16)

    with tile.TileContext(nc) as tc:
        with (
            tc.tile_pool(name="cst", bufs=1) as cst,
            tc.tile_pool(name="c2p", bufs=4) as c2p,
            tc.tile_pool(name="sp", bufs=4) as sp,
            tc.tile_pool(name="ps", bufs=1, space="PSUM") as ps,
            tc.tile_pool(name="d2ps", bufs=4, space="PSUM") as d2ps,
        ):
            t_g2 = cst.tile([P, pmax], F32)
            t_t1 = cst.tile([P, pmax], F32)
            t_gy2 = cst.tile([P, pmax], F32)
            t_p2 = cst.tile([P, 256], F32)
            t_yb = cst.tile([P, 256], F32)
            t_id = cst.tile([P, P], BF16)
            t_if = cst.tile([P, P], F32)
            t_ifn = cst.tile([P, P], F32)
            t_lsi = cst.tile([P, nidx], I16)
            t_lsd = cst.tile([P, nidx], BF16)
            nc.sync.dma_start(out=t_g2[:], in_=g2s[:])
            nc.sync.dma_start(out=t_t1[:], in_=t1s[:])
            nc.sync.dma_start(out=t_gy2[:], in_=gy2s[:])
            nc.sync.dma_start(out=t_p2[:], in_=p2c[:])
            nc.sync.dma_start(out=t_yb[:], in_=ybc[:])
            nc.sync.dma_start(out=t_id[:], in_=identb[:])
            nc.sync.dma_start(out=t_if[:], in_=identf[:])
            nc.sync.dma_start(out=t_ifn[:], in_=identfn[:])
            nc.sync.dma_start(out=t_lsi[:], in_=lsi[:])
            nc.sync.dma_start(out=t_lsd[:], in_=lsd[:])

            # ---- ring mask: scatter -> window roundtrip -> PE shift-accumulate
            t_pw = cst.tile([P, wfree], BF16)
            nc.gpsimd.local_scatter(
                t_pw[:], t_lsd[:], t_lsi[:],
                channels=P, num_elems=wfree, num_idxs=nidx,
            )
            nc.sync.dma_start(out=pwin[:], in_=t_pw[:])
            t_mega = cst.tile([P, na, 512], BF16)
            mega_in = bass.AP(pwin[:].tensor, 0, [[256, P], [256, na], [1, 512]])
            nc.sync.dma_start(out=t_mega[:], in_=mega_in)

            t_ring_ps = ps.tile([P, 256], F32, space="PSUM")
            n_off = len(offs_ab)
            for k, (a, b) in enumerate(offs_ab):
                nc.tensor.matmul(
                    t_ring_ps[:], t_id[:], t_mega[:, a, b:b + 256],
                    start=(k == 0), stop=(k == n_off - 1),
                )

            # ---- disk loop
            t_m = cst.tile([P, 256], F32)
            nc.vector.memset(t_m[:], 3.0e38)
            for j in range(pmax):
                t_c2 = c2p.tile([P, 256], F32)
                nc.scalar.activation(
                    t_c2[:], t_yb[:], mybir.ActivationFunctionType.Identity,
                    bias=t_t1[:, j:j + 1], scale=t_gy2[:, j:j + 1],
                )
                t_s = sp.tile([P, 256], F32)
                nc.gpsimd.tensor_scalar(
                    t_s[:], t_p2[:], t_g2[:, j:j + 1], None,
                    op0=mybir.AluOpType.add,
                )
                t_d2 = d2ps.tile([P, 256], F32, space="PSUM")
                nc.tensor.matmul(t_d2[:], t_if[:], t_s[:], start=True, stop=False)
                nc.tensor.matmul(t_d2[:], t_ifn[:], t_c2[:], start=False, stop=True)
                nc.vector.tensor_tensor(
                    out=t_m[:], in0=t_m[:], in1=t_d2[:],
                    op=mybir.AluOpType.min,
                )

            # ---- combine: flag = max((ring>=0.5)*2-1, -(M>=T2))
            t_f0 = cst.tile([P, 256], F32)
            nc.vector.tensor_scalar(
                t_f0[:], t_m[:], T2, -1.0,
                op0=mybir.AluOpType.is_ge, op1=mybir.AluOpType.mult,
            )
            t_r2 = cst.tile([P, 256], F32)
            nc.vector.tensor_scalar(
                t_r2[:], t_ring_ps[:], 0.5, 2.0,
                op0=mybir.AluOpType.is_ge, op1=mybir.AluOpType.mult,
            )
            t_fl = cst.tile([P, 256], F32)
            nc.vector.scalar_tensor_tensor(
                t_fl[:], t_r2[:], -1.0, t_f0[:],
                op0=mybir.AluOpType.add, op1=mybir.AluOpType.max,
            )
            nc.sync.dma_start(out=flag_out[:], in_=t_fl[:])

            # ---- pixels_out passthrough
            t_px = cst.tile([P, 512], F32)
            nc.sync.dma_start(out=t_px[:], in_=pxc[:])
            nc.sync.dma_start(out=px_out[:], in_=t_px[:])

    nc.compile()
    _prog_cache[key] = nc
    return nc


def kernel(images, gt_points, gt_nums):
    images = np.asarray(images)
    gt = np.asarray(gt_points, dtype=np.float32)
    gt_nums = np.asarray(gt_nums, dtype=np.int32)
    bs, _, h, w = images.shape
    assert (bs, h, w) == (BS, H, W)

    off = _ring_offsets()
    off_u = np.unique(off)                     # dedup: set-semantics scatter
    off_max = int(off_u.max())
    off_min = int(off_u.min())
    span = off_max - off_min
    na = span // 256 + 1
    wfree = int(np.ceil((256 * 127 + 256 * (na - 1) + 512) / P))
    wfree += wfree % 2
    offs_ab = []
    for o in off_u:
        s = off_max - int(o)
        offs_ab.append((s // 256, s % 256))

    # ---------- host prep per core ----------
    xs = np.arange(256, dtype=np.float32)
    ys = np.arange(256, dtype=np.float32)

    per_core = []
    row_lists = {}
    pmax = 1
    for c in range(N_CORES):
        b, x0 = c // 2, (c % 2) * P
        ng = int(gt_nums[b])
        gx = gt[b, :ng, 0]
        gy = gt[b, :ng, 1]
        g2 = (gx * gx + gy * gy).astype(np.float32)      # fl(fl(gx^2)+fl(gy^2))
        rows = []
        for p in range(P):
            x = np.float32(x0 + p)
            sel = np.abs(gx - x) <= R_CULL
            t1 = (np.float32(2.0) * (x * gx[sel])).astype(np.float32)  # 2*fl(x*gx)
            rows.append((g2[sel], t1, (np.float32(2.0) * gy[sel])))
            pmax = max(pmax, int(sel.sum()))
        row_lists[c] = rows

    pmax += pmax % 2

    for c in range(N_CORES):
        b, x0 = c // 2, (c % 2) * P
        ng = int(gt_nums[b])
        gx = gt[b, :ng, 0]
        gy = gt[b, :ng, 1]

        g2s = np.full((P, pmax), 1.0e30, np.float32)
        t1s = np.zeros((P, pmax), np.float32)
        gy2s = np.zeros((P, pmax), np.float32)
        for p, (g2r, t1r, gy2r) in enumerate(row_lists[c]):
            k = len(g2r)
            g2s[p, :k] = g2r
            t1s[p, :k] = t1r
            gy2s[p, :k] = gy2r

        # ring: base indices -> window slots
        base = np.round(gx * np.float32(W) + gy * np.float32(H)).astype(np.int64)
        n0 = x0 * 256
        win0 = n0 - off_max
        delta = np.unique(base - win0)
        delta = delta[(delta >= 0) & (delta < P * wfree)]
        part = (delta // wfree).astype(np.int64)
        pos = (delta % wfree).astype(np.int64)
        counts = np.bincount(part, minlength=P)
        nidx_c = int(counts.max()) if len(delta) else 1
        per_core.append(dict(b=b, x0=x0, g2s=g2s, t1s=t1s, gy2s=gy2s,
                             part=part, pos=pos, nidx=nidx_c))

    nidx = max(pc["nidx"] for pc in per_core)
    nidx += nidx % 2
    nidx = max(nidx, 2)

    # constants shared across cores except x-dependent P2/PXC
    ybc = np.broadcast_to(ys[None, :], (P, 256)).astype(np.float32).copy()
    identb = np.eye(P, dtype=ml_dtypes.bfloat16)
    identf_np = np.eye(P, dtype=np.float32)
    identfn_np = -np.eye(P, dtype=np.float32)
    lsd = np.ones((P, nidx), ml_dtypes.bfloat16)

    in_maps = []
    for c in range(N_CORES):
        pc = per_core[c]
        x0 = pc["x0"]
        xcol = (xs[x0:x0 + P])[:, None]
        p2c = (xcol * xcol + ys[None, :] * ys[None, :]).astype(np.float32)
        pxcol = np.empty((P, 256, 2), np.float32)
        pxcol[:, :, 0] = xcol
        pxcol[:, :, 1] = ys[None, :]
        lsi = np.full((P, nidx), -1, np.int16)
        fill = np.zeros(P, np.int64)
        for pt, po in zip(pc["part"], pc["pos"]):
            lsi[pt, fill[pt]] = po
            fill[pt] += 1
        in_maps.append(dict(
            g2s=pc["g2s"], t1s=pc["t1s"], gy2s=pc["gy2s"],
            p2c=p2c, ybc=ybc, pxc=pxcol.reshape(P, 512),
            identb=identb, identf=identf_np, identfn=identfn_np, lsi=lsi, lsd=lsd,
        ))

    nc = _build_program(pmax, nidx, wfree, na, offs_ab)
    res = run_bass_kernel_spmd(nc, in_maps, list(range(N_CORES)))

    flag = np.empty((BS, HW), np.float32)
    px = np.empty((BS, HW, 2), np.float32)
    for c in range(N_CORES):
        b, x0 = c // 2, (c % 2) * P
        r = res.results[c]
        flag[b, x0 * 256:(x0 + P) * 256] = r["flag_out"].reshape(-1)
        px[b, x0 * 256:(x0 + P) * 256] = r["px_out"].reshape(-1, 2)
    return px, flag


# revision 4
# speedup vs baseline: 1.1871x; 1.1871x over previous
"""Trainium2 Bass kernel for nn_NeighborPointsGenerator.

Data-parallel over (image, x-slab): 8 cores = 4 images x 2 x-slabs of 128 rows.
Per core:
  - disk/negative flag: per x-row point lists (host-binned, |gx-x|<=20.01),
    slot loop computing the reference's exact fp32 rounding chain:
      c2 = fl(2*gy*y + 2*fl(x*gx))      (ScalarE FMA, bit-exact vs XLA cpu)
      d2 = fl(fl(p2+g2) - c2)           (GPSIMD add + DVE subtract)
      M  = min(M, d2)                   (DVE)
    negative  <=>  M >= nextafter(nextafter(400))   (sqrt-free equivalence)
  - ring/positive flag: base indices scattered into a DRAM window
    (gpsimd local_scatter), reloaded via one overlapping 3D-AP DMA, and
    120 shifted slices accumulated on the PE into PSUM; ring = sum > 0.
  - flag = max(ring ? 1 : -1, -(M >= T2)) ; pixels_out copied from constants.
"""
import numpy as np
import ml_dtypes

import concourse.bass as bass
import concourse.bacc as bacc
import concourse.mybir as mybir
import concourse.tile as tile
from concourse.bass_utils import run_bass_kernel_spmd
from concourse._compat import get_trn_type

F32 = mybir.dt.float32
BF16 = mybir.dt.bfloat16
I16 = mybir.dt.int16

RADIUS, STRIDE, BASE_PTS = 5, 4, 8
BS, H, W = 4, 256, 256
HW = H * W
P = 128                  # partitions = x-rows per core
N_CORES = 8
R_CULL = 20.01           # |gx - x| beyond this can never give ref-d2 <= thr

_nextafter = np.nextafter
T2 = float(_nextafter(_nextafter(np.float32(400.0), np.float32(1e9)),
                      np.float32(1e9)))  # negative <=> min_d2 >= T2


def _ring_offsets():
    dxs, dys = [], []
    for i in range(RADIUS):
        r = (i + 1) * STRIDE
        n = BASE_PTS * (i + 1)
        ang = np.arange(n) / n * 2.0 * np.pi
        dxs.append(r * np.cos(ang))
        dys.append(r * np.sin(ang))
    dx = np.concatenate(dxs)
    dy = np.concatenate(dys)
    return np.round(dy * W + dx * H).astype(np.int32)  # (K,)


_prog_cache = {}


def _build_program(pmax, nidx, wfree, na, offs_ab):
    key = (pmax, nidx, wfree, na, tuple(offs_ab))
    if key in _prog_cache:
        return _prog_cache[key]

    nc = bacc.Bacc(get_trn_type() or "TRN2", target_bir_lowering=False, debug=True)

    g2s = nc.declare_dram_parameter("g2s", [P, pmax], F32, isOutput=False)
    t1s = nc.declare_dram_parameter("t1s", [P, pmax], F32, isOutput=False)
    gy2s = nc.declare_dram_parameter("gy2s", [P, pmax], F32, isOutput=False)
    p2c = nc.declare_dram_parameter("p2c", [P, 256], F32, isOutput=False)
    ybc = nc.declare_dram_parameter("ybc", [P, 256], F32, isOutput=False)
    pxc = nc.declare_dram_parameter("pxc", [P, 512], F32, isOutput=False)
    identb = nc.declare_dram_parameter("identb", [P, P], BF16, isOutput=False)
    identf = nc.declare_dram_parameter("identf", [P, P], F32, isOutput=False)
    identfn = nc.declare_dram_parameter("identfn", [P, P], F32, isOutput=False)
    lsi = nc.declare_dram_parameter("lsi", [P, nidx], I16, isOutput=False)
    lsd = nc.declare_dram_parameter("lsd", [P, nidx], BF16, isOutput=False)

    flag_out = nc.declare_dram_parameter("flag_out", [P, 256], F32, isOutput=True)
    px_out = nc.declare_dram_parameter("px_out", [P, 512], F32, isOutput=True)

    pwin = nc.dram_tensor("pwin", [P, wfree], BF16)

    with tile.TileContext(nc) as tc:
        with (
            tc.tile_pool(name="cst", bufs=1) as cst,
            tc.tile_pool(name="c2p", bufs=4) as c2p,
            tc.tile_pool(name="sp", bufs=4) as sp,
            tc.tile_pool(name="ps", bufs=1, space="PSUM") as ps,
            tc.tile_pool(name="d2p", bufs=4) as d2p,
        ):
            t_g2 = cst.tile([P, pmax], F32)
            t_t1 = cst.tile([P, pmax], F32)
            t_gy2 = cst.tile([P, pmax], F32)
            t_p2 = cst.tile([P, 256], F32)
            t_yb = cst.tile([P, 256], F32)
            t_id = cst.tile([P, P], BF16)
            t_if = cst.tile([P, P], F32)
            t_ifn = cst.tile([P, P], F32)
            t_lsi = cst.tile([P, nidx], I16)
            t_lsd = cst.tile([P, nidx], BF16)
            nc.sync.dma_start(out=t_g2[:], in_=g2s[:])
            nc.sync.dma_start(out=t_t1[:], in_=t1s[:])
            nc.sync.dma_start(out=t_gy2[:], in_=gy2s[:])
            nc.sync.dma_start(out=t_p2[:], in_=p2c[:])
            nc.sync.dma_start(out=t_yb[:], in_=ybc[:])
            nc.sync.dma_start(out=t_id[:], in_=identb[:])
            nc.sync.dma_start(out=t_if[:], in_=identf[:])
            nc.sync.dma_start(out=t_ifn[:], in_=identfn[:])
            nc.sync.dma_start(out=t_lsi[:], in_=lsi[:])
            nc.sync.dma_start(out=t_lsd[:], in_=lsd[:])

            # ---- ring mask: scatter -> window roundtrip -> PE shift-accumulate
            t_pw = cst.tile([P, wfree], BF16)
            nc.gpsimd.local_scatter(
                t_pw[:], t_lsd[:], t_lsi[:],
                channels=P, num_elems=wfree, num_idxs=nidx,
            )
            nc.sync.dma_start(out=pwin[:], in_=t_pw[:])
            t_mega = cst.tile([P, na, 512], BF16)
            mega_in = bass.AP(pwin[:].tensor, 0, [[256, P], [256, na], [1, 512]])
            nc.sync.dma_start(out=t_mega[:], in_=mega_in)

            t_ring_ps = ps.tile([P, 256], F32, space="PSUM")
            n_off = len(offs_ab)
            for k, (a, b) in enumerate(offs_ab):
                nc.tensor.matmul(
                    t_ring_ps[:], t_id[:], t_mega[:, a, b:b + 256],
                    start=(k == 0), stop=(k == n_off - 1),
                )

            # ---- disk loop
            t_m = cst.tile([P, 256], F32)
            nc.vector.memset(t_m[:], 3.0e38)
            for j in range(pmax):
                t_c2 = c2p.tile([P, 256], F32)
                nc.scalar.activation(
                    t_c2[:], t_yb[:], mybir.ActivationFunctionType.Identity,
                    bias=t_t1[:, j:j + 1], scale=t_gy2[:, j:j + 1],
                )
                t_s = sp.tile([P, 256], F32)
                nc.scalar.activation(
                    t_s[:], t_p2[:], mybir.ActivationFunctionType.Identity,
                    bias=t_g2[:, j:j + 1], scale=1.0,
                )
                t_d2 = d2p.tile([P, 256], F32)
                nc.gpsimd.tensor_tensor(
                    out=t_d2[:], in0=t_s[:], in1=t_c2[:],
                    op=mybir.AluOpType.subtract,
                )
                nc.vector.tensor_tensor(
                    out=t_m[:], in0=t_m[:], in1=t_d2[:],
                    op=mybir.AluOpType.min,
                )

            # ---- combine: flag = max((ring>=0.5)*2-1, -(M>=T2))
            t_f0 = cst.tile([P, 256], F32)
            nc.vector.tensor_scalar(
                t_f0[:], t_m[:], T2, -1.0,
                op0=mybir.AluOpType.is_ge, op1=mybir.AluOpType.mult,
            )
            t_r2 = cst.tile([P, 256], F32)
            nc.vector.tensor_scalar(
                t_r2[:], t_ring_ps[:], 0.5, 2.0,
                op0=mybir.AluOpType.is_ge, op1=mybir.AluOpType.mult,
            )
            t_fl = cst.tile([P, 256], F32)
            nc.vector.scalar_tensor_tensor(
                t_fl[:], t_r2[:], -1.0, t_f0[:],
                op0=mybir.AluOpType.add, op1=mybir.AluOpType.max,
            )
            nc.sync.dma_start(out=flag_out[:], in_=t_fl[:])

            # ---- pixels_out passthrough
            t_px = cst.tile([P, 512], F32)
            nc.sync.dma_start(out=t_px[:], in_=pxc[:])
            nc.sync.dma_start(out=px_out[:], in_=t_px[:])

    nc.compile()
    _prog_cache[key] = nc
    return nc


def kernel(images, gt_points, gt_nums):
    images = np.asarray(images)
    gt = np.asarray(gt_points, dtype=np.float32)
    gt_nums = np.asarray(gt_nums, dtype=np.int32)
    bs, _, h, w = images.shape
    assert (bs, h, w) == (BS, H, W)

    off = _ring_offsets()
    off_u = np.unique(off)                     # dedup: set-semantics scatter
    off_max = int(off_u.max())
    off_min = int(off_u.min())
    span = off_max - off_min
    na = span // 256 + 1
    wfree = int(np.ceil((256 * 127 + 256 * (na - 1) + 512) / P))
    wfree += wfree % 2
    offs_ab = []
    for o in off_u:
        s = off_max - int(o)
        offs_ab.append((s // 256, s % 256))

    # ---------- host prep per core ----------
    xs = np.arange(256, dtype=np.float32)
    ys = np.arange(256, dtype=np.float32)

    per_core = []
    row_lists = {}
    pmax = 1
    for c in range(N_CORES):
        b, x0 = c // 2, (c % 2) * P
        ng = int(gt_nums[b])
        gx = gt[b, :ng, 0]
        gy = gt[b, :ng, 1]
        g2 = (gx * gx + gy * gy).astype(np.float32)      # fl(fl(gx^2)+fl(gy^2))
        rows = []
        for p in range(P):
            x = np.float32(x0 + p)
            sel = np.abs(gx - x) <= R_CULL
            t1 = (np.float32(2.0) * (x * gx[sel])).astype(np.float32)  # 2*fl(x*gx)
            rows.append((g2[sel], t1, (np.float32(2.0) * gy[sel])))
            pmax = max(pmax, int(sel.sum()))
        row_lists[c] = rows

    pmax += pmax % 2

    for c in range(N_CORES):
        b, x0 = c // 2, (c % 2) * P
        ng = int(gt_nums[b])
        gx = gt[b, :ng, 0]
        gy = gt[b, :ng, 1]

        g2s = np.full((P, pmax), 1.0e30, np.float32)
        t1s = np.zeros((P, pmax), np.float32)
        gy2s = np.zeros((P, pmax), np.float32)
        for p, (g2r, t1r, gy2r) in enumerate(row_lists[c]):
            k = len(g2r)
            g2s[p, :k] = g2r
            t1s[p, :k] = t1r
            gy2s[p, :k] = gy2r

        # ring: base indices -> window slots
        base = np.round(gx * np.float32(W) + gy * np.float32(H)).astype(np.int64)
        n0 = x0 * 256
        win0 = n0 - off_max
        delta = np.unique(base - win0)
        delta = delta[(delta >= 0) & (delta < P * wfree)]
        part = (delta // wfree).astype(np.int64)
        pos = (delta % wfree).astype(np.int64)
        counts = np.bincount(part, minlength=P)
        nidx_c = int(counts.max()) if len(delta) else 1
        per_core.append(dict(b=b, x0=x0, g2s=g2s, t1s=t1s, gy2s=gy2s,
                             part=part, pos=pos, nidx=nidx_c))

    nidx = max(pc["nidx"] for pc in per_core)
    nidx += nidx % 2
    nidx = max(nidx, 2)

    # constants shared across cores except x-dependent P2/PXC
    ybc = np.broadcast_to(ys[None, :], (P, 256)).astype(np.float32).copy()
    identb = np.eye(P, dtype=ml_dtypes.bfloat16)
    identf_np = np.eye(P, dtype=np.float32)
    identfn_np = -np.eye(P, dtype=np.float32)
    lsd = np.ones((P, nidx), ml_dtypes.bfloat16)

    in_maps = []
    for c in range(N_CORES):
        pc = per_core[c]
        x0 = pc["x0"]
        xcol = (xs[x0:x0 + P])[:, None]
        p2c = (xcol * xcol + ys[None, :] * ys[None, :]).astype(np.float32)
        pxcol = np.empty((P, 256, 2), np.float32)
        pxcol[:, :, 0] = xcol
        pxcol[:, :, 1] = ys[None, :]
        lsi = np.full((P, nidx), -1, np.int16)
        fill = np.zeros(P, np.int64)
        for pt, po in zip(pc["part"], pc["pos"]):
            lsi[pt, fill[pt]] = po
            fill[pt] += 1
        in_maps.append(dict(
            g2s=pc["g2s"], t1s=pc["t1s"], gy2s=pc["gy2s"],
            p2c=p2c, ybc=ybc, pxc=pxcol.reshape(P, 512),
            identb=identb, identf=identf_np, identfn=identfn_np, lsi=lsi, lsd=lsd,
        ))

    nc = _build_program(pmax, nidx, wfree, na, offs_ab)
    res = run_bass_kernel_spmd(nc, in_maps, list(range(N_CORES)))

    flag = np.empty((BS, HW), np.float32)
    px = np.empty((BS, HW, 2), np.float32)
    for c in range(N_CORES):
        b, x0 = c // 2, (c % 2) * P
        r = res.results[c]
        flag[b, x0 * 256:(x0 + P) * 256] = r["flag_out"].reshape(-1)
        px[b, x0 * 256:(x0 + P) * 256] = r["px_out"].reshape(-1, 2)
    return px, flag


# revision 5
# speedup vs baseline: 1.4794x; 1.2462x over previous
"""Trainium2 Bass kernel for nn_NeighborPointsGenerator.

Data-parallel over (image, x-slab): 8 cores = 4 images x 2 x-slabs of 128 rows.
Per core:
  - disk/negative flag: per x-row point lists (host-binned, |gx-x|<=20.01),
    slot loop computing the reference's exact fp32 rounding chain:
      c2 = fl(2*gy*y + 2*fl(x*gx))      (ScalarE FMA, bit-exact vs XLA cpu)
      d2 = fl(fl(p2+g2) - c2)           (GPSIMD add + DVE subtract)
      M  = min(M, d2)                   (DVE)
    negative  <=>  M >= nextafter(nextafter(400))   (sqrt-free equivalence)
  - ring/positive flag: base indices scattered into a DRAM window
    (gpsimd local_scatter), reloaded via one overlapping 3D-AP DMA, and
    120 shifted slices accumulated on the PE into PSUM; ring = sum > 0.
  - flag = max(ring ? 1 : -1, -(M >= T2)) ; pixels_out copied from constants.
"""
import numpy as np
import ml_dtypes

import concourse.bass as bass
import concourse.bacc as bacc
import concourse.mybir as mybir
import concourse.tile as tile
from concourse.bass_utils import run_bass_kernel_spmd
from concourse._compat import get_trn_type

F32 = mybir.dt.float32
BF16 = mybir.dt.bfloat16
I16 = mybir.dt.int16

RADIUS, STRIDE, BASE_PTS = 5, 4, 8
BS, H, W = 4, 256, 256
HW = H * W
P = 128                  # partitions = x-rows per core
N_CORES = 8
R_CULL = 20.01           # |gx - x| beyond this can never give ref-d2 <= thr

_nextafter = np.nextafter
T2 = float(_nextafter(_nextafter(np.float32(400.0), np.float32(1e9)),
                      np.float32(1e9)))  # negative <=> min_d2 >= T2


def _ring_offsets():
    dxs, dys = [], []
    for i in range(RADIUS):
        r = (i + 1) * STRIDE
        n = BASE_PTS * (i + 1)
        ang = np.arange(n) / n * 2.0 * np.pi
        dxs.append(r * np.cos(ang))
        dys.append(r * np.sin(ang))
    dx = np.concatenate(dxs)
    dy = np.concatenate(dys)
    return np.round(dy * W + dx * H).astype(np.int32)  # (K,)


_prog_cache = {}


def _build_program(pmax, nidx, wfree, na, offs_ab):
    key = (pmax, nidx, wfree, na, tuple(offs_ab))
    if key in _prog_cache:
        return _prog_cache[key]

    nc = bacc.Bacc(get_trn_type() or "TRN2", target_bir_lowering=False, debug=True)

    g2s = nc.declare_dram_parameter("g2s", [P, pmax], F32, isOutput=False)
    t1s = nc.declare_dram_parameter("t1s", [P, pmax], F32, isOutput=False)
    gy2s = nc.declare_dram_parameter("gy2s", [P, pmax], F32, isOutput=False)
    p2c = nc.declare_dram_parameter("p2c", [P, 256], F32, isOutput=False)
    ybc = nc.declare_dram_parameter("ybc", [P, 256], F32, isOutput=False)
    pxc = nc.declare_dram_parameter("pxc", [P, 512], F32, isOutput=False)
    identb = nc.declare_dram_parameter("identb", [P, P], BF16, isOutput=False)
    identf = nc.declare_dram_parameter("identf", [P, P], F32, isOutput=False)
    identfn = nc.declare_dram_parameter("identfn", [P, P], F32, isOutput=False)
    lsi = nc.declare_dram_parameter("lsi", [P, nidx], I16, isOutput=False)
    lsd = nc.declare_dram_parameter("lsd", [P, nidx], BF16, isOutput=False)

    flag_out = nc.declare_dram_parameter("flag_out", [P, 256], F32, isOutput=True)
    px_out = nc.declare_dram_parameter("px_out", [P, 512], F32, isOutput=True)

    pwin = nc.dram_tensor("pwin", [P, wfree], BF16)

    with tile.TileContext(nc) as tc:
        with (
            tc.tile_pool(name="cst", bufs=1) as cst,
            tc.tile_pool(name="c2p", bufs=4) as c2p,
            tc.tile_pool(name="sp", bufs=4) as sp,
            tc.tile_pool(name="ps", bufs=1, space="PSUM") as ps,
            tc.tile_pool(name="d2p", bufs=4) as d2p,
        ):
            t_g2 = cst.tile([P, pmax], F32)
            t_t1 = cst.tile([P, pmax], F32)
            t_gy2 = cst.tile([P, pmax], F32)
            t_p2 = cst.tile([P, 256], F32)
            t_yb = cst.tile([P, 256], F32)
            t_id = cst.tile([P, P], BF16)
            t_if = cst.tile([P, P], F32)
            t_ifn = cst.tile([P, P], F32)
            t_lsi = cst.tile([P, nidx], I16)
            t_lsd = cst.tile([P, nidx], BF16)
            nc.sync.dma_start(out=t_g2[:], in_=g2s[:])
            nc.sync.dma_start(out=t_t1[:], in_=t1s[:])
            nc.sync.dma_start(out=t_gy2[:], in_=gy2s[:])
            nc.sync.dma_start(out=t_p2[:], in_=p2c[:])
            nc.sync.dma_start(out=t_yb[:], in_=ybc[:])
            nc.sync.dma_start(out=t_id[:], in_=identb[:])
            nc.sync.dma_start(out=t_if[:], in_=identf[:])
            nc.sync.dma_start(out=t_ifn[:], in_=identfn[:])
            nc.sync.dma_start(out=t_lsi[:], in_=lsi[:])
            nc.sync.dma_start(out=t_lsd[:], in_=lsd[:])

            # ---- ring mask: scatter -> window roundtrip -> PE shift-accumulate
            t_pw = cst.tile([P, wfree], BF16)
            nc.gpsimd.local_scatter(
                t_pw[:], t_lsd[:], t_lsi[:],
                channels=P, num_elems=wfree, num_idxs=nidx,
            )
            nc.sync.dma_start(out=pwin[:], in_=t_pw[:])
            t_mega = cst.tile([P, na, 512], BF16)
            mega_in = bass.AP(pwin[:].tensor, 0, [[256, P], [256, na], [1, 512]])
            nc.sync.dma_start(out=t_mega[:], in_=mega_in)

            t_ring_ps = ps.tile([P, 256], F32, space="PSUM")
            n_off = len(offs_ab)
            for k, (a, b) in enumerate(offs_ab):
                nc.tensor.matmul(
                    t_ring_ps[:], t_id[:], t_mega[:, a, b:b + 256],
                    start=(k == 0), stop=(k == n_off - 1),
                )

            # ---- disk loop
            t_m = cst.tile([P, 256], F32)
            nc.vector.memset(t_m[:], 3.0e38)
            for j in range(pmax):
                t_c2 = c2p.tile([P, 256], F32)
                nc.scalar.activation(
                    t_c2[:], t_yb[:], mybir.ActivationFunctionType.Identity,
                    bias=t_t1[:, j:j + 1], scale=t_gy2[:, j:j + 1],
                )
                t_s = sp.tile([P, 256], F32)
                nc.vector.tensor_scalar(
                    t_s[:], t_p2[:], t_g2[:, j:j + 1], None,
                    op0=mybir.AluOpType.add,
                )
                t_d2 = d2p.tile([P, 256], F32)
                nc.gpsimd.tensor_tensor(
                    out=t_d2[:], in0=t_s[:], in1=t_c2[:],
                    op=mybir.AluOpType.subtract,
                )
                nc.vector.tensor_tensor(
                    out=t_m[:], in0=t_m[:], in1=t_d2[:],
                    op=mybir.AluOpType.min,
                )

            # ---- combine: flag = max((ring>=0.5)*2-1, -(M>=T2))
            t_f0 = cst.tile([P, 256], F32)
            nc.vector.tensor_scalar(
                t_f0[:], t_m[:], T2, -1.0,
                op0=mybir.AluOpType.is_ge, op1=mybir.AluOpType.mult,
            )
            t_r2 = cst.tile([P, 256], F32)
            nc.vector.tensor_scalar(
                t_r2[:], t_ring_ps[:], 0.5, 2.0,
                op0=mybir.AluOpType.is_ge, op1=mybir.AluOpType.mult,
            )
            t_fl = cst.tile([P, 256], F32)
            nc.vector.scalar_tensor_tensor(
                t_fl[:], t_r2[:], -1.0, t_f0[:],
                op0=mybir.AluOpType.add, op1=mybir.AluOpType.max,
            )
            nc.sync.dma_start(out=flag_out[:], in_=t_fl[:])

            # ---- pixels_out passthrough
            t_px = cst.tile([P, 512], F32)
            nc.sync.dma_start(out=t_px[:], in_=pxc[:])
            nc.sync.dma_start(out=px_out[:], in_=t_px[:])

    nc.compile()
    _prog_cache[key] = nc
    return nc


def kernel(images, gt_points, gt_nums):
    images = np.asarray(images)
    gt = np.asarray(gt_points, dtype=np.float32)
    gt_nums = np.asarray(gt_nums, dtype=np.int32)
    bs, _, h, w = images.shape
    assert (bs, h, w) == (BS, H, W)

    off = _ring_offsets()
    off_u = np.unique(off)                     # dedup: set-semantics scatter
    off_max = int(off_u.max())
    off_min = int(off_u.min())
    span = off_max - off_min
    na = span // 256 + 1
    wfree = int(np.ceil((256 * 127 + 256 * (na - 1) + 512) / P))
    wfree += wfree % 2
    offs_ab = []
    for o in off_u:
        s = off_max - int(o)
        offs_ab.append((s // 256, s % 256))

    # ---------- host prep per core ----------
    xs = np.arange(256, dtype=np.float32)
    ys = np.arange(256, dtype=np.float32)

    per_core = []
    row_lists = {}
    pmax = 1
    for c in range(N_CORES):
        b, x0 = c // 2, (c % 2) * P
        ng = int(gt_nums[b])
        gx = gt[b, :ng, 0]
        gy = gt[b, :ng, 1]
        g2 = (gx * gx + gy * gy).astype(np.float32)      # fl(fl(gx^2)+fl(gy^2))
        rows = []
        for p in range(P):
            x = np.float32(x0 + p)
            sel = np.abs(gx - x) <= R_CULL
            t1 = (np.float32(2.0) * (x * gx[sel])).astype(np.float32)  # 2*fl(x*gx)
            rows.append((g2[sel], t1, (np.float32(2.0) * gy[sel])))
            pmax = max(pmax, int(sel.sum()))
        row_lists[c] = rows

    pmax += pmax % 2

    for c in range(N_CORES):
        b, x0 = c // 2, (c % 2) * P
        ng = int(gt_nums[b])
        gx = gt[b, :ng, 0]
        gy = gt[b, :ng, 1]

        g2s = np.full((P, pmax), 1.0e30, np.float32)
        t1s = np.zeros((P, pmax), np.float32)
        gy2s = np.zeros((P, pmax), np.float32)
        for p, (g2r, t1r, gy2r) in enumerate(row_lists[c]):
            k = len(g2r)
            g2s[p, :k] = g2r
            t1s[p, :k] = t1r
            gy2s[p, :k] = gy2r

        # ring: base indices -> window slots
        base = np.round(gx * np.float32(W) + gy * np.float32(H)).astype(np.int64)
        n0 = x0 * 256
        win0 = n0 - off_max
        delta = np.unique(base - win0)
        delta = delta[(delta >= 0) & (delta < P * wfree)]
        part = (delta // wfree).astype(np.int64)
        pos = (delta % wfree).astype(np.int64)
        counts = np.bincount(part, minlength=P)
        nidx_c = int(counts.max()) if len(delta) else 1
        per_core.append(dict(b=b, x0=x0, g2s=g2s, t1s=t1s, gy2s=gy2s,
                             part=part, pos=pos, nidx=nidx_c))

    nidx = max(pc["nidx"] for pc in per_core)
    nidx += nidx % 2
    nidx = max(nidx, 2)

    # constants shared across cores except x-dependent P2/PXC
    ybc = np.broadcast_to(ys[None, :], (P, 256)).astype(np.float32).copy()
    identb = np.eye(P, dtype=ml_dtypes.bfloat16)
    identf_np = np.eye(P, dtype=np.float32)
    identfn_np = -np.eye(P, dtype=np.float32)
    lsd = np.ones((P, nidx), ml_dtypes.bfloat16)

    in_maps = []
    for c in range(N_CORES):
        pc = per_core[c]
        x0 = pc["x0"]
        xcol = (xs[x0:x0 + P])[:, None]
        p2c = (xcol * xcol + ys[None, :] * ys[None, :]).astype(np.float32)
        pxcol = np.empty((P, 256, 2), np.float32)
        pxcol[:, :, 0] = xcol
        pxcol[:, :, 1] = ys[None, :]
        lsi = np.full((P, nidx), -1, np.int16)
        fill = np.zeros(P, np.int64)
        for pt, po in zip(pc["part"], pc["pos"]):
            lsi[pt, fill[pt]] = po
            fill[pt] += 1
        in_maps.append(dict(
            g2s=pc["g2s"], t1s=pc["t1s"], gy2s=pc["gy2s"],
            p2c=p2c, ybc=ybc, pxc=pxcol.reshape(P, 512),
            identb=identb, identf=identf_np, identfn=identfn_np, lsi=lsi, lsd=lsd,
        ))

    nc = _build_program(pmax, nidx, wfree, na, offs_ab)
    res = run_bass_kernel_spmd(nc, in_maps, list(range(N_CORES)))

    flag = np.empty((BS, HW), np.float32)
    px = np.empty((BS, HW, 2), np.float32)
    for c in range(N_CORES):
        b, x0 = c // 2, (c % 2) * P
        r = res.results[c]
        flag[b, x0 * 256:(x0 + P) * 256] = r["flag_out"].reshape(-1)
        px[b, x0 * 256:(x0 + P) * 256] = r["px_out"].reshape(-1, 2)
    return px, flag


# revision 6
# speedup vs baseline: 1.9380x; 1.3100x over previous
"""Trainium2 Bass kernel for nn_NeighborPointsGenerator.

Data-parallel over (image, x-slab): 8 cores = 4 images x 2 x-slabs of 128 rows.
Per core:
  - disk/negative flag: per (x-row, y-half) point lists (host-binned,
    |gx-x|<=20.01 and gy within the half +-20.6), slot loop computing the
    reference's exact fp32 rounding chain:
      c2 = fl(2*gy*y + 2*fl(x*gx))      (ScalarE FMA, bit-exact vs XLA cpu)
      s  = fl(p2 + g2)                  (DVE tensor_scalar)
      d2 = fl(s - c2)                   (GPSIMD subtract, 8 slots batched)
      M  = min(M, d2)                   (DVE, 8 slots batched)
    negative  <=>  M >= nextafter(nextafter(400))   (sqrt-free equivalence)
  - ring/positive flag: base indices scattered into a DRAM window
    (gpsimd local_scatter), reloaded via one overlapping 3D-AP DMA, and
    ~61 shifted slices accumulated on the PE into PSUM; ring = sum > 0.
  - flag = max(ring ? 1 : -1, -(M >= T2)) ; pixels_out copied from constants.
"""
import numpy as np
import ml_dtypes

import concourse.bass as bass
import concourse.bacc as bacc
import concourse.mybir as mybir
import concourse.tile as tile
from concourse.bass_utils import run_bass_kernel_spmd
from concourse._compat import get_trn_type

F32 = mybir.dt.float32
BF16 = mybir.dt.bfloat16
I16 = mybir.dt.int16

RADIUS, STRIDE, BASE_PTS = 5, 4, 8
BS, H, W = 4, 256, 256
HW = H * W
P = 128                  # partitions = x-rows per core
N_CORES = 8
R_CULL = 20.01           # beyond this a point can never give ref-d2 <= thr
Y_SLACK = 0.6
NB = 8                   # slots batched per GPSIMD subtract / DVE min

_nextafter = np.nextafter
T2 = float(_nextafter(_nextafter(np.float32(400.0), np.float32(1e9)),
                      np.float32(1e9)))  # negative <=> min_d2 >= T2


def _ring_offsets():
    dxs, dys = [], []
    for i in range(RADIUS):
        r = (i + 1) * STRIDE
        n = BASE_PTS * (i + 1)
        ang = np.arange(n) / n * 2.0 * np.pi
        dxs.append(r * np.cos(ang))
        dys.append(r * np.sin(ang))
    dx = np.concatenate(dxs)
    dy = np.concatenate(dys)
    return np.round(dy * W + dx * H).astype(np.int32)  # (K,)


_prog_cache = {}


def _build_program(pmax, nidx, wfree, na, offs_ab):
    """pmax = padded max slots per (row, y-half); multiple of NB."""
    key = (pmax, nidx, wfree, na, tuple(offs_ab))
    if key in _prog_cache:
        return _prog_cache[key]

    nc = bacc.Bacc(get_trn_type() or "TRN2", target_bir_lowering=False, debug=True)

    # slab arrays: halves stacked along free dim: [P, 2*pmax]
    g2s = nc.declare_dram_parameter("g2s", [P, 2 * pmax], F32, isOutput=False)
    t1s = nc.declare_dram_parameter("t1s", [P, 2 * pmax], F32, isOutput=False)
    gy2s = nc.declare_dram_parameter("gy2s", [P, 2 * pmax], F32, isOutput=False)
    p2c = nc.declare_dram_parameter("p2c", [P, 256], F32, isOutput=False)
    ybc = nc.declare_dram_parameter("ybc", [P, 256], F32, isOutput=False)
    pxc = nc.declare_dram_parameter("pxc", [P, 512], F32, isOutput=False)
    identb = nc.declare_dram_parameter("identb", [P, P], BF16, isOutput=False)
    lsi = nc.declare_dram_parameter("lsi", [P, nidx], I16, isOutput=False)
    lsd = nc.declare_dram_parameter("lsd", [P, nidx], BF16, isOutput=False)

    flag_out = nc.declare_dram_parameter("flag_out", [P, 256], F32, isOutput=True)
    px_out = nc.declare_dram_parameter("px_out", [P, 512], F32, isOutput=True)

    pwin = nc.dram_tensor("pwin", [P, wfree], BF16)

    HB = 128  # half width
    n_oct = pmax // NB

    with tile.TileContext(nc) as tc:
        with (
            tc.tile_pool(name="cst", bufs=1) as cst,
            tc.tile_pool(name="c2p", bufs=3) as c2p,
            tc.tile_pool(name="sp", bufs=3) as sp,
            tc.tile_pool(name="d2p", bufs=3) as d2p,
            tc.tile_pool(name="ps", bufs=1, space="PSUM") as ps,
        ):
            t_g2 = cst.tile([P, 2 * pmax], F32)
            t_t1 = cst.tile([P, 2 * pmax], F32)
            t_gy2 = cst.tile([P, 2 * pmax], F32)
            t_p2 = cst.tile([P, 256], F32)
            t_yb = cst.tile([P, 256], F32)
            t_id = cst.tile([P, P], BF16)
            t_lsi = cst.tile([P, nidx], I16)
            t_lsd = cst.tile([P, nidx], BF16)
            nc.sync.dma_start(out=t_g2[:], in_=g2s[:])
            nc.sync.dma_start(out=t_t1[:], in_=t1s[:])
            nc.sync.dma_start(out=t_gy2[:], in_=gy2s[:])
            nc.sync.dma_start(out=t_p2[:], in_=p2c[:])
            nc.sync.dma_start(out=t_yb[:], in_=ybc[:])
            nc.sync.dma_start(out=t_id[:], in_=identb[:])
            nc.sync.dma_start(out=t_lsi[:], in_=lsi[:])
            nc.sync.dma_start(out=t_lsd[:], in_=lsd[:])

            # ---- ring mask: scatter -> window roundtrip -> PE shift-accumulate
            t_pw = cst.tile([P, wfree], BF16)
            nc.gpsimd.local_scatter(
                t_pw[:], t_lsd[:], t_lsi[:],
                channels=P, num_elems=wfree, num_idxs=nidx,
            )
            nc.sync.dma_start(out=pwin[:], in_=t_pw[:])
            t_mega = cst.tile([P, na, 512], BF16)
            mega_in = bass.AP(pwin[:].tensor, 0, [[256, P], [256, na], [1, 512]])
            nc.sync.dma_start(out=t_mega[:], in_=mega_in)

            t_ring_ps = ps.tile([P, 256], F32, space="PSUM")
            n_off = len(offs_ab)
            for k, (a, b) in enumerate(offs_ab):
                nc.tensor.matmul(
                    t_ring_ps[:], t_id[:], t_mega[:, a, b:b + 256],
                    start=(k == 0), stop=(k == n_off - 1),
                )

            # ---- disk loop: y-halves, NB-slot batches
            t_mw0 = cst.tile([P, NB * HB], F32)
            t_mw1 = cst.tile([P, NB * HB], F32)
            t_mw = [t_mw0, t_mw1]
            nc.vector.memset(t_mw0[:], 3.0e38)
            nc.vector.memset(t_mw1[:], 3.0e38)
            for hh in range(2):
                ysl = slice(hh * HB, (hh + 1) * HB)
                base = hh * pmax
                for q in range(n_oct):
                    t_c2 = c2p.tile([P, NB * HB], F32)
                    t_s = sp.tile([P, NB * HB], F32)
                    for k in range(NB):
                        j = base + q * NB + k
                        ksl = slice(k * HB, (k + 1) * HB)
                        nc.scalar.activation(
                            t_c2[:, ksl], t_yb[:, ysl],
                            mybir.ActivationFunctionType.Identity,
                            bias=t_t1[:, j:j + 1], scale=t_gy2[:, j:j + 1],
                        )
                        nc.vector.tensor_scalar(
                            t_s[:, ksl], t_p2[:, ysl], t_g2[:, j:j + 1], None,
                            op0=mybir.AluOpType.add,
                        )
                    t_d2 = d2p.tile([P, NB * HB], F32)
                    nc.gpsimd.tensor_tensor(
                        out=t_d2[:], in0=t_s[:], in1=t_c2[:],
                        op=mybir.AluOpType.subtract,
                    )
                    nc.vector.tensor_tensor(
                        out=t_mw[hh][:], in0=t_mw[hh][:], in1=t_d2[:],
                        op=mybir.AluOpType.min,
                    )

            # fold NB-wide accumulators to [P, 128] each, into t_m
            t_m = cst.tile([P, 256], F32)
            t_fold = cst.tile([P, NB * HB // 2], F32)
            for hh in range(2):
                cur = t_mw[hh]
                w = NB * HB
                while w > HB:
                    half = w // 2
                    if half == HB:
                        dst = t_m[:, hh * HB:(hh + 1) * HB]
                    else:
                        dst = t_fold[:, :half]
                    nc.vector.tensor_tensor(
                        out=dst, in0=cur[:, :half], in1=cur[:, half:w],
                        op=mybir.AluOpType.min,
                    )
                    cur = t_fold
                    w = half

            # ---- combine: flag = max((ring>=0.5)*2-1, -(M>=T2))
            t_f0 = cst.tile([P, 256], F32)
            nc.vector.tensor_scalar(
                t_f0[:], t_m[:], T2, -1.0,
                op0=mybir.AluOpType.is_ge, op1=mybir.AluOpType.mult,
            )
            t_r2 = cst.tile([P, 256], F32)
            nc.vector.tensor_scalar(
                t_r2[:], t_ring_ps[:], 0.5, 2.0,
                op0=mybir.AluOpType.is_ge, op1=mybir.AluOpType.mult,
            )
            t_fl = cst.tile([P, 256], F32)
            nc.vector.scalar_tensor_tensor(
                t_fl[:], t_r2[:], -1.0, t_f0[:],
                op0=mybir.AluOpType.add, op1=mybir.AluOpType.max,
            )
            nc.sync.dma_start(out=flag_out[:], in_=t_fl[:])

            # ---- pixels_out passthrough
            t_px = cst.tile([P, 512], F32)
            nc.sync.dma_start(out=t_px[:], in_=pxc[:])
            nc.sync.dma_start(out=px_out[:], in_=t_px[:])

    nc.compile()
    _prog_cache[key] = nc
    return nc


def kernel(images, gt_points, gt_nums):
    images = np.asarray(images)
    gt = np.asarray(gt_points, dtype=np.float32)
    gt_nums = np.asarray(gt_nums, dtype=np.int32)
    bs, _, h, w = images.shape
    assert (bs, h, w) == (BS, H, W)

    off = _ring_offsets()
    off_u = np.unique(off)                     # dedup: set-semantics scatter
    off_max = int(off_u.max())
    off_min = int(off_u.min())
    span = off_max - off_min
    na = span // 256 + 1
    wfree = int(np.ceil((256 * 127 + 256 * (na - 1) + 512) / P))
    wfree += wfree % 2
    offs_ab = []
    for o in off_u:
        s = off_max - int(o)
        offs_ab.append((s // 256, s % 256))

    xs = np.arange(256, dtype=np.float32)
    ys = np.arange(256, dtype=np.float32)
    HB = 128

    # ---------- host prep per core ----------
    per_core = []
    row_lists = {}
    pmax = 1
    for c in range(N_CORES):
        b, x0 = c // 2, (c % 2) * P
        ng = int(gt_nums[b])
        gx = gt[b, :ng, 0]
        gy = gt[b, :ng, 1]
        g2 = (gx * gx + gy * gy).astype(np.float32)
        rows = []
        for p in range(P):
            x = np.float32(x0 + p)
            selx = np.abs(gx - x) <= R_CULL
            per_half = []
            for hh in range(2):
                y0, y1 = hh * HB, (hh + 1) * HB
                sel = selx & (gy > y0 - R_CULL - Y_SLACK) \
                           & (gy < y1 - 1 + R_CULL + Y_SLACK)
                t1 = (np.float32(2.0) * (x * gx[sel])).astype(np.float32)
                per_half.append((g2[sel], t1,
                                 (np.float32(2.0) * gy[sel])))
                pmax = max(pmax, int(sel.sum()))
            rows.append(per_half)
        row_lists[c] = rows

    pmax = ((pmax + NB - 1) // NB) * NB

    for c in range(N_CORES):
        b, x0 = c // 2, (c % 2) * P
        ng = int(gt_nums[b])
        gx = gt[b, :ng, 0]
        gy = gt[b, :ng, 1]

        g2s = np.full((P, 2 * pmax), 1.0e30, np.float32)
        t1s = np.zeros((P, 2 * pmax), np.float32)
        gy2s = np.zeros((P, 2 * pmax), np.float32)
        for p in range(P):
            for hh in range(2):
                g2r, t1r, gy2r = row_lists[c][p][hh]
                k = len(g2r)
                o = hh * pmax
                g2s[p, o:o + k] = g2r
                t1s[p, o:o + k] = t1r
                gy2s[p, o:o + k] = gy2r

        # ring: base indices -> window slots
        base = np.round(gx * np.float32(W) + gy * np.float32(H)).astype(np.int64)
        n0 = x0 * 256
        win0 = n0 - off_max
        delta = np.unique(base - win0)
        delta = delta[(delta >= 0) & (delta < P * wfree)]
        part = (delta // wfree).astype(np.int64)
        pos = (delta % wfree).astype(np.int64)
        counts = np.bincount(part, minlength=P)
        nidx_c = int(counts.max()) if len(delta) else 1
        per_core.append(dict(b=b, x0=x0, g2s=g2s, t1s=t1s, gy2s=gy2s,
                             part=part, pos=pos, nidx=nidx_c))

    nidx = max(pc["nidx"] for pc in per_core)
    nidx += nidx % 2
    nidx = max(nidx, 2)

    ybc = np.broadcast_to(ys[None, :], (P, 256)).astype(np.float32).copy()
    identb = np.eye(P, dtype=ml_dtypes.bfloat16)
    lsd = np.ones((P, nidx), ml_dtypes.bfloat16)

    in_maps = []
    for c in range(N_CORES):
        pc = per_core[c]
        x0 = pc["x0"]
        xcol = (xs[x0:x0 + P])[:, None]
        p2c = (xcol * xcol + ys[None, :] * ys[None, :]).astype(np.float32)
        pxcol = np.empty((P, 256, 2), np.float32)
        pxcol[:, :, 0] = xcol
        pxcol[:, :, 1] = ys[None, :]
        lsi = np.full((P, nidx), -1, np.int16)
        fill = np.zeros(P, np.int64)
        for pt, po in zip(pc["part"], pc["pos"]):
            lsi[pt, fill[pt]] = po
            fill[pt] += 1
        in_maps.append(dict(
            g2s=pc["g2s"], t1s=pc["t1s"], gy2s=pc["gy2s"],
            p2c=p2c, ybc=ybc, pxc=pxcol.reshape(P, 512),
            identb=identb, lsi=lsi, lsd=lsd,
        ))

    nc = _build_program(pmax, nidx, wfree, na, offs_ab)
    res = run_bass_kernel_spmd(nc, in_maps, list(range(N_CORES)))

    flag = np.empty((BS, HW), np.float32)
    px = np.empty((BS, HW, 2), np.float32)
    for c in range(N_CORES):
        b, x0 = c // 2, (c % 2) * P
        r = res.results[c]
        flag[b, x0 * 256:(x0 + P) * 256] = r["flag_out"].reshape(-1)
        px[b, x0 * 256:(x0 + P) * 256] = r["px_out"].reshape(-1, 2)
    return px, flag
